# revision 4
# baseline (speedup 1.0000x reference)
"""Causal self-attention (B=2, T=2048, D=768, H=12) on 8 TRN2 cores.

Sharding: core r handles batch b=r%2 and head-group g=r//2 (3 heads).
Replica groups are parity-matched ({0,2,4,6} batch 0, {1,3,5,7} batch 1):
the observed core-boot skew is odd/even structured, so each 4-core group
syncs only among cores that start together, and the start barrier (an
AllGather over the merged replica groups) never couples the two groups.

Per query block bi (512 tokens), fully pipelined:
  1. qkv projection for the block (this core's 3 heads).
  2. attention: S^T = K.Q with keys on partitions; heads 0/1 run
     concurrently on PE row groups via tile_position, head 2 pairs
     consecutive j-tiles the same way (q/k for head 2 are duplicated
     into both partition halves to satisfy tile_position alignment).
     exp(S/8) without max-subtraction; denominator via a ones-row
     appended to V in the O^T = V_aug^T E matmul.
  3. normalize O^T straight out of PSUM (reciprocal on the [1,512]
     denominator row, partition-broadcast, multiply).
  4. partial output projection over this core's 192 features for all
     512 tokens -> y_part [512, 768] fp16.
  5. 4-core ReduceScatter(add): core at group position c receives the
     summed y for tokens [512*bi + 128*c, +128).
The RS for block bi overlaps qkv+attention of block bi+1; only the last
block's proj + RS is exposed.  No 8-core sync anywhere.
"""

import numpy as np

import concourse.bass as bass
import concourse.bacc as bacc
import concourse.mybir as mybir
import concourse.tile as tile
from concourse.bass_utils import run_bass_kernel_spmd

F32 = mybir.dt.float32
F16 = mybir.dt.float16

B, T, D = 2, 2048, 768
H, DH = 12, 64
NCORES = 8
HPC = H // 4          # heads per core = 3
QK = HPC * DH         # 192 rows of q (or k) per core
KC = D // 128         # 6 contraction chunks
NBI = T // 512        # 4 query blocks
TBLK = 128            # tokens of y per core per block after RS

RG = [[0, 2, 4, 6], [1, 3, 5, 7]]

EXP_SCALE = 1.0 / np.sqrt(DH)  # 0.125
VW = 65               # v_aug column stride (64 + ones row)


def _emit(tc, aps):
    nc = tc.nc
    xT, wqkT, wvT, wpT, triu, y = (
        aps["xT"], aps["wqkT"], aps["wvT"], aps["wpT"], aps["triu"], aps["y"])

    pools = []

    def pool(name, bufs, space="SBUF"):
        p = tc.tile_pool(name=name, bufs=bufs, space=space)
        pools.append(p)
        return p.__enter__()

    consts = pool("consts", 1)
    xw = pool("xw", 1)
    qk_sb = pool("qk_sb", 1)
    v_sb = pool("v_sb", 1)
    work = pool("work", 4)
    norm = pool("norm", 2)
    ot_sb = pool("ot_sb", 2)
    ysb = pool("ysb", 2)
    dram = pool("dram", 1, space="DRAM")
    ps = pool("ps", 3, space="PSUM")
    ps_o = pool("ps_o", 1, space="PSUM")
    ps_pj = pool("ps_pj", 1, space="PSUM")

    # ---- input loads: first-block x columns first so qkv(0) starts ASAP
    triu_sb = consts.tile([128, 128], F16, tag="triu", name="triu")
    nc.gpsimd.dma_start(triu_sb[:], triu[:, :])

    xT_sb = [xw.tile([128, T], F16, tag=f"xT{k}", name=f"xT{k}") for k in range(KC)]
    wqk_sb = [consts.tile([128, 2 * QK], F16, tag=f"wqk{k}", name=f"wqk{k}") for k in range(KC)]
    wv_sb = [consts.tile([128, QK], F16, tag=f"wv{k}", name=f"wv{k}") for k in range(KC)]
    x_engs = [nc.sync, nc.scalar, nc.sync]
    for k in range(KC):
        nc.gpsimd.dma_start(wqk_sb[k][:], wqkT[k * 128:(k + 1) * 128, :])
        x_engs[k % 3].dma_start(xT_sb[k][:, 0:512], xT[k * 128:(k + 1) * 128, 0:512])
    for k in range(KC):
        nc.gpsimd.dma_start(wv_sb[k][:], wvT[k * 128:(k + 1) * 128, :])
        x_engs[k % 3].dma_start(xT_sb[k][:, 512:T], xT[k * 128:(k + 1) * 128, 512:T])
    wpA_sb = consts.tile([128, D], F16, tag="wpA", name="wpA")
    wpB_sb = consts.tile([64, D], F16, tag="wpB", name="wpB")
    nc.gpsimd.dma_start(wpA_sb[:], wpT[0:128, :])
    nc.gpsimd.dma_start(wpB_sb[:], wpT[128:QK, :])

    # ---- persistent q/k/v tiles ----
    # heads 0/1 packed into [128, T] (rows 0-63 / 64-127); head 2 q/k are
    # duplicated into both halves so j-tile pairs can use PE row groups.
    qTp = qk_sb.tile([128, T], F16, tag="qTp", name="qTp")
    kTp = qk_sb.tile([128, T], F16, tag="kTp", name="kTp")
    qT2 = qk_sb.tile([128, T], F16, tag="qT2", name="qT2")
    kT2 = qk_sb.tile([128, T], F16, tag="kT2", name="kT2")
    v_aug = [v_sb.tile([128, (T // 128) * VW], F16, tag=f"v{h}", name=f"v{h}")
             for h in range(HPC)]

    # DRAM staging for the per-block ReduceScatter
    rs_in = [dram.tile([512, D], F16, tag=f"rsi{bi}", name=f"rsi{bi}")
             for bi in range(NBI)]
    rs_out = [dram.tile([TBLK, D], F16, tag=f"rso{bi}", name=f"rso{bi}")
              for bi in range(NBI)]

    def emit_qkv(bi):
        ns = slice(bi * 512, (bi + 1) * 512)
        for m in range(3):
            p = ps.tile([128, 512], F32, tag="ps", name="qkps")
            for k in range(KC):
                nc.tensor.matmul(
                    p[:],
                    wqk_sb[k][:, m * 128:(m + 1) * 128],
                    xT_sb[k][:, ns],
                    start=(k == 0), stop=(k == KC - 1))
            if m == 0:
                nc.vector.tensor_copy(qTp[:, ns], p[:])
            elif m == 1:
                nc.vector.tensor_copy(qT2[0:64, ns], p[0:64, :])
                nc.vector.tensor_copy(qT2[64:128, ns], p[0:64, :])
                nc.vector.tensor_copy(kTp[0:64, ns], p[64:128, :])
            else:
                nc.vector.tensor_copy(kTp[64:128, ns], p[0:64, :])
                nc.vector.tensor_copy(kT2[0:64, ns], p[64:128, :])
                nc.vector.tensor_copy(kT2[64:128, ns], p[64:128, :])
        for tt in range(bi * 4, bi * 4 + 4):
            p = ps.tile([128, 512], F32, tag="ps", name="vps")
            for k in range(KC):
                nc.tensor.matmul(
                    p[:, 0:QK],
                    xT_sb[k][:, tt * 128:(tt + 1) * 128],
                    wv_sb[k][:],
                    start=(k == 0), stop=(k == KC - 1))
            for h in range(HPC):
                nc.vector.tensor_copy(
                    v_aug[h][:, tt * VW:tt * VW + 64], p[:, h * 64:(h + 1) * 64])
                nc.vector.memset(v_aug[h][:, tt * VW + 64:tt * VW + VW], 1.0)

    def s_pair(o_ps01, o_ps2, bi, tj, ntj):
        """S + exp + mask + AV for heads 0/1 at j-tile tj (concurrent on
        PE row groups), and nothing for head 2 (done in pairs below)."""
        dtile = tj - 4 * bi
        lo = max(dtile, 0) * 128
        js = slice(tj * 128, (tj + 1) * 128)
        qs = slice(bi * 512 + lo, (bi + 1) * 512)
        pss = []
        for h in range(2):
            p = ps.tile([128, 512], F32, tag="ps", name=f"s{h}")
            nc.tensor.matmul(
                p[:, lo:], kTp[64 * h:64 * h + 64, js], qTp[64 * h:64 * h + 64, qs],
                start=True, stop=True, tile_position=(64 * h, 0))
            pss.append(p)
        for h in range(2):
            e = work.tile([128, 512], F16, tag="e", name=f"e{h}")
            nc.scalar.activation(
                e[:, lo:], pss[h][:, lo:],
                mybir.ActivationFunctionType.Exp, scale=EXP_SCALE)
            if dtile >= 0:
                nc.vector.tensor_mul(
                    e[:, lo:lo + 128], e[:, lo:lo + 128], triu_sb[:])
            nc.tensor.matmul(
                o_ps01[h][:, lo:],
                v_aug[h][:, tj * VW:(tj + 1) * VW],
                e[:, lo:],
                start=(tj == 0), stop=(tj == ntj - 1))

    def s2_pair(o_ps2, bi, tj0, ntj):
        """head 2: j-tiles tj0/tj0+1 concurrently via PE row groups."""
        pair = [tj for tj in (tj0, tj0 + 1) if tj < ntj]
        pss = []
        los = []
        for idx, tj in enumerate(pair):
            dtile = tj - 4 * bi
            lo = max(dtile, 0) * 128
            los.append(lo)
            js = slice(tj * 128, (tj + 1) * 128)
            qs = slice(bi * 512 + lo, (bi + 1) * 512)
            p = ps.tile([128, 512], F32, tag="ps", name=f"s2_{idx}")
            nc.tensor.matmul(
                p[:, lo:], kT2[64 * idx:64 * idx + 64, js],
                qT2[64 * idx:64 * idx + 64, qs],
                start=True, stop=True, tile_position=(64 * idx, 0))
            pss.append(p)
        for idx, tj in enumerate(pair):
            lo = los[idx]
            e = work.tile([128, 512], F16, tag="e", name=f"e2_{idx}")
            nc.scalar.activation(
                e[:, lo:], pss[idx][:, lo:],
                mybir.ActivationFunctionType.Exp, scale=EXP_SCALE)
            if tj - 4 * bi >= 0:
                nc.vector.tensor_mul(
                    e[:, lo:lo + 128], e[:, lo:lo + 128], triu_sb[:])
            nc.tensor.matmul(
                o_ps2[:, lo:],
                v_aug[2][:, tj * VW:(tj + 1) * VW],
                e[:, lo:],
                start=(tj == 0), stop=(tj == ntj - 1))

    y_engs = [nc.sync, nc.gpsimd, nc.scalar, nc.sync]

    for bi in range(NBI):
        emit_qkv(bi)
        ntj = 4 * bi + 4
        o01 = [ps_o.tile([65, 512], F32, tag=f"o{h}", name=f"o{h}") for h in range(2)]
        o2 = ps_o.tile([65, 512], F32, tag="o2", name="o2")
        for tj in range(ntj):
            s_pair(o01, o2, bi, tj, ntj)
        for tj0 in range(0, ntj, 2):
            s2_pair(o2, bi, tj0, ntj)

        # ---- normalize straight out of PSUM into OT tiles ----
        o_all = o01 + [o2]
        OT01 = ot_sb.tile([128, 512], F16, tag="OT01", name="OT01")
        OT2 = ot_sb.tile([64, 512], F16, tag="OT2", name="OT2")
        dsts = [OT01[0:64], OT01[64:128], OT2[:, :]]
        for h in range(HPC):
            rec = norm.tile([1, 512], F32, tag="rec", name="rec")
            nc.vector.reciprocal(rec[:], o_all[h][64:65, :])
            rb = norm.tile([64, 512], F32, tag="rb", name="rb")
            nc.gpsimd.partition_broadcast(rb[:], rec[:])
            nc.vector.tensor_mul(dsts[h], o_all[h][0:64, :], rb[:])

        # ---- partial proj over this core's 192 features ----
        for tt in range(4):
            ts = slice(tt * 128, (tt + 1) * 128)
            y_t = ysb.tile([128, D], F16, tag="y_t", name="y_t")
            for on, osz, tag in ((0, 512, "pjA"), (512, 256, "pjB")):
                pj = ps_pj.tile([128, osz], F32, tag=tag, name=tag)
                nc.tensor.matmul(
                    pj[:], OT01[:, ts], wpA_sb[:, on:on + osz],
                    start=True, stop=False)
                nc.tensor.matmul(
                    pj[:], OT2[:, ts], wpB_sb[:, on:on + osz],
                    start=False, stop=True)
                nc.vector.tensor_copy(y_t[:, on:on + osz], pj[:])
            y_engs[tt].dma_start(rs_in[bi][tt * 128:(tt + 1) * 128, :], y_t[:])

        nc.gpsimd.collective_compute(
            "ReduceScatter",
            mybir.AluOpType.add,
            replica_groups=RG,
            ins=[rs_in[bi].opt()],
            outs=[rs_out[bi].opt()],
        )
        y_engs[bi % 4].dma_start(
            y[bi * TBLK:(bi + 1) * TBLK, :], rs_out[bi][:, :])

    for p in reversed(pools):
        p.__exit__(None, None, None)


_NC_CACHE = {}


def _get_nc():
    if "nc" in _NC_CACHE:
        return _NC_CACHE["nc"]
    nc = bacc.Bacc("TRN2", num_devices=NCORES, debug=False)
    aps = {
        "xT": nc.dram_tensor("xT", [D, T], F16, kind="ExternalInput").ap(),
        "wqkT": nc.dram_tensor("wqkT", [D, 2 * QK], F16, kind="ExternalInput").ap(),
        "wvT": nc.dram_tensor("wvT", [D, QK], F16, kind="ExternalInput").ap(),
        "wpT": nc.dram_tensor("wpT", [QK, D], F16, kind="ExternalInput").ap(),
        "triu": nc.dram_tensor("triu", [128, 128], F16, kind="ExternalInput").ap(),
        "y": nc.dram_tensor("y", [NBI * TBLK, D], F16, kind="ExternalOutput").ap(),
    }
    with tile.TileContext(nc, num_cores=NCORES) as tc:
        _emit(tc, aps)
    nc.compile()
    _NC_CACHE["nc"] = nc
    return nc


def make_in_maps(x, W_qkv, W_proj):
    triu = np.triu(np.ones((128, 128), dtype=np.float16))
    wpT_full = np.ascontiguousarray(W_proj.T).astype(np.float16)  # [in, out]
    in_maps = []
    for r in range(NCORES):
        b, g = r % 2, r // 2
        rs = slice(QK * g, QK * (g + 1))
        wq = W_qkv[0:D][rs]
        wk = W_qkv[D:2 * D][rs]
        wv = W_qkv[2 * D:3 * D][rs]
        wqkT = np.ascontiguousarray(
            np.concatenate([wq, wk], axis=0).T).astype(np.float16)
        wvT = np.ascontiguousarray(wv.T).astype(np.float16)
        wpT = np.ascontiguousarray(wpT_full[rs, :])
        xT = np.ascontiguousarray(x[b].T).astype(np.float16)
        in_maps.append({"xT": xT, "wqkT": wqkT, "wvT": wvT,
                        "wpT": wpT, "triu": triu})
    return in_maps


def assemble(results):
    y = np.empty((B, T, D), dtype=np.float32)
    for r in range(NCORES):
        b, c = r % 2, r // 2
        yr = results[r]["y"]
        for bi in range(NBI):
            y[b, bi * 512 + c * TBLK: bi * 512 + (c + 1) * TBLK, :] = (
                yr[bi * TBLK:(bi + 1) * TBLK, :].astype(np.float32))
    return y


def kernel(**inputs):
    x = np.asarray(inputs["x"], dtype=np.float32)
    W_qkv = np.asarray(inputs["W_qkv"], dtype=np.float32)
    W_proj = np.asarray(inputs["W_proj"], dtype=np.float32)
    nc = _get_nc()
    in_maps = make_in_maps(x, W_qkv, W_proj)
    res = run_bass_kernel_spmd(nc, in_maps, core_ids=list(range(NCORES)))
    return assemble(res.results)


# revision 9
# speedup vs baseline: 1.1763x; 1.1763x over previous
"""Causal self-attention (B=2, T=2048, D=768, H=12) on 8 TRN2 cores.

Sharding: core r handles batch b=r%2 and head-group g=r//2 (3 heads).
Replica groups are parity-matched ({0,2,4,6} batch 0, {1,3,5,7} batch 1):
observed core-boot skew is odd/even structured, so each 4-core group
syncs only among cores that boot together, and the auto start-barrier
(AllGather over merged replica groups) never couples the two groups.

Per query block bi (512 tokens), fully pipelined:
  1. qkv projection for the block (this core's 3 heads).
  2. attention: S^T = K.Q with keys on partitions, heads 0/1 sharing one
     [128,1024] PSUM strip (one exp ACT covers both), head 2 pairing
     consecutive j-tiles the same way.  The AV matmul for j-tile tj-1 is
     emitted after the S matmuls for tj, so the in-order PE never waits
     on the scalar-engine exp.  exp(S/8) without max-subtraction;
     denominator via a ones-row appended to V (O^T = V_aug^T E).
  3. normalize O^T out of PSUM: scalar-ACT Reciprocal on the [1,512]
     denominator row (DVE reciprocal is free-size-bound and 5x slower),
     gpsimd partition-broadcast, vector multiply.
  4. partial output projection over this core's 192 features for all
     512 tokens -> y_part [512, 768] fp16.
  5. 4-core ReduceScatter(add): the core at group position c receives
     the summed y for tokens [512*bi + 128*c, +128) directly.
The RS for block bi overlaps qkv+attention of block bi+1; only the last
block's proj + RS is exposed.  No 8-core sync anywhere.
"""

import numpy as np

import concourse.bass as bass
import concourse.bacc as bacc
import concourse.mybir as mybir
import concourse.tile as tile
from concourse.bass_utils import run_bass_kernel_spmd

F32 = mybir.dt.float32
F16 = mybir.dt.float16

B, T, D = 2, 2048, 768
H, DH = 12, 64
NCORES = 8
HPC = H // 4          # heads per core = 3
QK = HPC * DH         # 192 rows of q (or k) per core
KC = D // 128         # 6 contraction chunks
NBI = T // 512        # 4 query blocks
TBLK = 128            # tokens of y per core per block after RS

RG = [[0, 2, 4, 6], [1, 3, 5, 7]]

EXP_SCALE = 1.0 / np.sqrt(DH)  # 0.125
VW = 65               # v_aug column stride (64 + ones row)


def _emit(tc, aps):
    nc = tc.nc
    xT, wqkT, wvT, wpT, triu, y = (
        aps["xT"], aps["wqkT"], aps["wvT"], aps["wpT"], aps["triu"], aps["y"])

    pools = []

    def pool(name, bufs, space="SBUF"):
        p = tc.tile_pool(name=name, bufs=bufs, space=space)
        pools.append(p)
        return p.__enter__()

    consts = pool("consts", 1)
    xw = pool("xw", 1)
    qk_sb = pool("qk_sb", 1)
    v_sb = pool("v_sb", 1)
    work = pool("work", 3)
    norm = pool("norm", 2)
    ot_sb = pool("ot_sb", 2)
    ysb = pool("ysb", 2)
    dram = pool("dram", 1, space="DRAM")
    ps = pool("ps", 2, space="PSUM")
    ps_o = pool("ps_o", 1, space="PSUM")
    ps_pj = pool("ps_pj", 1, space="PSUM")

    # ---- input loads: first-block x columns first so qkv(0) starts ASAP
    triu_sb = consts.tile([128, 128], F16, tag="triu", name="triu")
    nc.gpsimd.dma_start(triu_sb[:], triu[:, :])

    xT_sb = [xw.tile([128, T], F16, tag=f"xT{k}", name=f"xT{k}") for k in range(KC)]
    wqk_sb = [consts.tile([128, 2 * QK], F16, tag=f"wqk{k}", name=f"wqk{k}") for k in range(KC)]
    wv_sb = [consts.tile([128, QK], F16, tag=f"wv{k}", name=f"wv{k}") for k in range(KC)]
    x_engs = [nc.sync, nc.scalar]
    for k in range(KC):
        nc.gpsimd.dma_start(wqk_sb[k][:], wqkT[k * 128:(k + 1) * 128, :])
        x_engs[k % 2].dma_start(xT_sb[k][:, 0:512], xT[k * 128:(k + 1) * 128, 0:512])
    for k in range(KC):
        nc.gpsimd.dma_start(wv_sb[k][:], wvT[k * 128:(k + 1) * 128, :])
        x_engs[k % 2].dma_start(xT_sb[k][:, 512:T], xT[k * 128:(k + 1) * 128, 512:T])
    wpA_sb = consts.tile([128, D], F16, tag="wpA", name="wpA")
    wpB_sb = consts.tile([64, D], F16, tag="wpB", name="wpB")
    nc.gpsimd.dma_start(wpA_sb[:], wpT[0:128, :])
    nc.gpsimd.dma_start(wpB_sb[:], wpT[128:QK, :])

    # ---- persistent q/k/v tiles ----
    # heads 0/1 packed into [128, T] (rows 0-63 / 64-127); head 2 in [64, T].
    qTp = qk_sb.tile([128, T], F16, tag="qTp", name="qTp")
    kTp = qk_sb.tile([128, T], F16, tag="kTp", name="kTp")
    qT2 = qk_sb.tile([64, T], F16, tag="qT2", name="qT2")
    kT2 = qk_sb.tile([64, T], F16, tag="kT2", name="kT2")
    v_aug = [v_sb.tile([128, (T // 128) * VW], F16, tag=f"v{h}", name=f"v{h}")
             for h in range(HPC)]

    # DRAM staging for the per-block ReduceScatter
    rs_in = [dram.tile([512, D], F16, tag=f"rsi{bi}", name=f"rsi{bi}")
             for bi in range(NBI)]
    rs_out = [dram.tile([TBLK, D], F16, tag=f"rso{bi}", name=f"rso{bi}")
              for bi in range(NBI)]

    def emit_qkv(bi):
        ns = slice(bi * 512, (bi + 1) * 512)
        for m in range(3):
            p = ps.tile([128, 1024], F32, tag="s", name="qkps")[:, 0:512]
            for k in range(KC):
                nc.tensor.matmul(
                    p[:],
                    wqk_sb[k][:, m * 128:(m + 1) * 128],
                    xT_sb[k][:, ns],
                    start=(k == 0), stop=(k == KC - 1))
            if m == 0:
                nc.vector.tensor_copy(qTp[:, ns], p[:])
            elif m == 1:
                nc.vector.tensor_copy(qT2[:, ns], p[0:64, :])
                nc.vector.tensor_copy(kTp[0:64, ns], p[64:128, :])
            else:
                nc.vector.tensor_copy(kTp[64:128, ns], p[0:64, :])
                nc.vector.tensor_copy(kT2[:, ns], p[64:128, :])
        for tt in range(bi * 4, bi * 4 + 4):
            p = ps.tile([128, 1024], F32, tag="s", name="vps")[:, 0:512]
            for k in range(KC):
                nc.tensor.matmul(
                    p[:, 0:QK],
                    xT_sb[k][:, tt * 128:(tt + 1) * 128],
                    wv_sb[k][:],
                    start=(k == 0), stop=(k == KC - 1))
            for h in range(HPC):
                nc.vector.tensor_copy(
                    v_aug[h][:, tt * VW:tt * VW + 64], p[:, h * 64:(h + 1) * 64])
                nc.vector.memset(v_aug[h][:, tt * VW + 64:tt * VW + VW], 1.0)

    y_engs = [nc.sync, nc.gpsimd, nc.scalar, nc.sync]

    for bi in range(NBI):
        emit_qkv(bi)
        ntj = 4 * bi + 4
        o01 = [ps_o.tile([65, 512], F32, tag=f"o{h}", name=f"o{h}") for h in range(2)]
        o2 = ps_o.tile([65, 512], F32, tag="o2", name="o2")

        # ---- heads 0/1: S(tj) then AV(tj-1), one exp per j-tile ----
        pend = None  # (e_tile, tj, lo)

        def flush_av(o_pair, ntj_):
            e, tj, lo = pend
            for h in range(2):
                if tj - 4 * bi >= 0:
                    nc.vector.tensor_mul(
                        e[:, h * 512 + lo:h * 512 + lo + 128],
                        e[:, h * 512 + lo:h * 512 + lo + 128], triu_sb[:])
                nc.tensor.matmul(
                    o_pair[h][:, lo:],
                    v_aug[h][:, tj * VW:(tj + 1) * VW],
                    e[:, h * 512 + lo:(h + 1) * 512],
                    start=(tj == 0), stop=(tj == ntj_ - 1))

        for tj in range(ntj):
            dtile = tj - 4 * bi
            lo = max(dtile, 0) * 128
            js = slice(tj * 128, (tj + 1) * 128)
            qs = slice(bi * 512 + lo, (bi + 1) * 512)
            s_ps = ps.tile([128, 1024], F32, tag="s", name="s")
            nc.tensor.matmul(s_ps[:, lo:512], kTp[0:64, js], qTp[0:64, qs],
                             start=True, stop=True)
            nc.tensor.matmul(s_ps[:, 512 + lo:1024], kTp[64:128, js], qTp[64:128, qs],
                             start=True, stop=True)
            e = work.tile([128, 1024], F16, tag="e", name="e")
            if lo == 0:
                nc.scalar.activation(e[:], s_ps[:],
                                     mybir.ActivationFunctionType.Exp, scale=EXP_SCALE)
            else:
                for h in range(2):
                    nc.scalar.activation(
                        e[:, h * 512 + lo:(h + 1) * 512],
                        s_ps[:, h * 512 + lo:(h + 1) * 512],
                        mybir.ActivationFunctionType.Exp, scale=EXP_SCALE)
            if pend is not None:
                flush_av(o01, ntj)
            pend = (e, tj, lo)
        flush_av(o01, ntj)

        # ---- head 2: paired j-tiles, AV one pair behind ----
        pend2 = None  # (e, pair, los)
        for tj0 in range(0, ntj, 2):
            pair = (tj0, tj0 + 1)
            s_ps = ps.tile([128, 1024], F32, tag="s", name="s2")
            e = work.tile([128, 1024], F16, tag="e", name="e2")
            los = []
            for idx, tj in enumerate(pair):
                lo = max(tj - 4 * bi, 0) * 128
                los.append(lo)
                js = slice(tj * 128, (tj + 1) * 128)
                qs = slice(bi * 512 + lo, (bi + 1) * 512)
                nc.tensor.matmul(
                    s_ps[:, idx * 512 + lo:(idx + 1) * 512], kT2[:, js], qT2[:, qs],
                    start=True, stop=True)
            if los[1] == 0:
                nc.scalar.activation(e[:], s_ps[:],
                                     mybir.ActivationFunctionType.Exp, scale=EXP_SCALE)
            else:
                for idx in range(2):
                    nc.scalar.activation(
                        e[:, idx * 512 + los[idx]:(idx + 1) * 512],
                        s_ps[:, idx * 512 + los[idx]:(idx + 1) * 512],
                        mybir.ActivationFunctionType.Exp, scale=EXP_SCALE)
            if pend2 is not None:
                ep, pp, losp = pend2
                for idx, tj in enumerate(pp):
                    if tj - 4 * bi >= 0:
                        nc.vector.tensor_mul(
                            ep[:, idx * 512 + losp[idx]:idx * 512 + losp[idx] + 128],
                            ep[:, idx * 512 + losp[idx]:idx * 512 + losp[idx] + 128],
                            triu_sb[:])
                    nc.tensor.matmul(
                        o2[:, losp[idx]:],
                        v_aug[2][:, tj * VW:(tj + 1) * VW],
                        ep[:, idx * 512 + losp[idx]:(idx + 1) * 512],
                        start=(tj == 0), stop=(tj == ntj - 1))
            pend2 = (e, pair, los)
        ep, pp, losp = pend2
        for idx, tj in enumerate(pp):
            if tj - 4 * bi >= 0:
                nc.vector.tensor_mul(
                    ep[:, idx * 512 + losp[idx]:idx * 512 + losp[idx] + 128],
                    ep[:, idx * 512 + losp[idx]:idx * 512 + losp[idx] + 128],
                    triu_sb[:])
            nc.tensor.matmul(
                o2[:, losp[idx]:],
                v_aug[2][:, tj * VW:(tj + 1) * VW],
                ep[:, idx * 512 + losp[idx]:(idx + 1) * 512],
                start=(tj == 0), stop=(tj == ntj - 1))

        # ---- normalize straight out of PSUM into OT tiles ----
        o_all = o01 + [o2]
        OT01 = ot_sb.tile([128, 512], F16, tag="OT01", name="OT01")
        OT2 = ot_sb.tile([64, 512], F16, tag="OT2", name="OT2")
        dsts = [OT01[0:64], OT01[64:128], OT2[:, :]]
        for h in range(HPC):
            # 1/den as exp(-ln(den)) on the scalar engine: DVE reciprocal
            # is free-size-bound (~6.5ns/col) and 5x slower than two ACTs.
            lnd = norm.tile([1, 512], F32, tag="lnd", name="lnd")
            nc.scalar.activation(lnd[:], o_all[h][64:65, :],
                                 mybir.ActivationFunctionType.Ln)
            rec = norm.tile([1, 512], F32, tag="rec", name="rec")
            nc.scalar.activation(rec[:], lnd[:],
                                 mybir.ActivationFunctionType.Exp, scale=-1.0)
            rb = norm.tile([64, 512], F32, tag="rb", name="rb")
            nc.gpsimd.partition_broadcast(rb[:], rec[:])
            nc.vector.tensor_mul(dsts[h], o_all[h][0:64, :], rb[:])

        # ---- partial proj over this core's 192 features ----
        for tt in range(4):
            ts = slice(tt * 128, (tt + 1) * 128)
            y_t = ysb.tile([128, D], F16, tag="y_t", name="y_t")
            for on, osz in ((0, 512), (512, 256)):
                pj = ps_pj.tile([128, 512], F32, tag="pj", name="pj")
                nc.tensor.matmul(
                    pj[:, 0:osz], OT01[:, ts], wpA_sb[:, on:on + osz],
                    start=True, stop=False)
                nc.tensor.matmul(
                    pj[:, 0:osz], OT2[:, ts], wpB_sb[:, on:on + osz],
                    start=False, stop=True)
                nc.vector.tensor_copy(y_t[:, on:on + osz], pj[:, 0:osz])
            y_engs[tt].dma_start(rs_in[bi][tt * 128:(tt + 1) * 128, :], y_t[:])

        nc.gpsimd.collective_compute(
            "ReduceScatter",
            mybir.AluOpType.add,
            replica_groups=RG,
            ins=[rs_in[bi].opt()],
            outs=[rs_out[bi].opt()],
        )
        y_engs[bi % 2].dma_start(
            y[bi * TBLK:(bi + 1) * TBLK, :], rs_out[bi][:, :])

    for p in reversed(pools):
        p.__exit__(None, None, None)


_NC_CACHE = {}


def _get_nc():
    if "nc" in _NC_CACHE:
        return _NC_CACHE["nc"]
    nc = bacc.Bacc("TRN2", num_devices=NCORES, debug=False)
    aps = {
        "xT": nc.dram_tensor("xT", [D, T], F16, kind="ExternalInput").ap(),
        "wqkT": nc.dram_tensor("wqkT", [D, 2 * QK], F16, kind="ExternalInput").ap(),
        "wvT": nc.dram_tensor("wvT", [D, QK], F16, kind="ExternalInput").ap(),
        "wpT": nc.dram_tensor("wpT", [QK, D], F16, kind="ExternalInput").ap(),
        "triu": nc.dram_tensor("triu", [128, 128], F16, kind="ExternalInput").ap(),
        "y": nc.dram_tensor("y", [NBI * TBLK, D], F16, kind="ExternalOutput").ap(),
    }
    with tile.TileContext(nc, num_cores=NCORES) as tc:
        _emit(tc, aps)
    nc.compile()
    _NC_CACHE["nc"] = nc
    return nc


def make_in_maps(x, W_qkv, W_proj):
    triu = np.triu(np.ones((128, 128), dtype=np.float16))
    wpT_full = np.ascontiguousarray(W_proj.T).astype(np.float16)  # [in, out]
    in_maps = []
    for r in range(NCORES):
        b, g = r % 2, r // 2
        rs = slice(QK * g, QK * (g + 1))
        wq = W_qkv[0:D][rs]
        wk = W_qkv[D:2 * D][rs]
        wv = W_qkv[2 * D:3 * D][rs]
        wqkT = np.ascontiguousarray(
            np.concatenate([wq, wk], axis=0).T).astype(np.float16)
        wvT = np.ascontiguousarray(wv.T).astype(np.float16)
        wpT = np.ascontiguousarray(wpT_full[rs, :])
        xT = np.ascontiguousarray(x[b].T).astype(np.float16)
        in_maps.append({"xT": xT, "wqkT": wqkT, "wvT": wvT,
                        "wpT": wpT, "triu": triu})
    return in_maps


def assemble(results):
    y = np.empty((B, T, D), dtype=np.float32)
    for r in range(NCORES):
        b, c = r % 2, r // 2
        yr = results[r]["y"]
        for bi in range(NBI):
            y[b, bi * 512 + c * TBLK: bi * 512 + (c + 1) * TBLK, :] = (
                yr[bi * TBLK:(bi + 1) * TBLK, :].astype(np.float32))
    return y


def kernel(**inputs):
    x = np.asarray(inputs["x"], dtype=np.float32)
    W_qkv = np.asarray(inputs["W_qkv"], dtype=np.float32)
    W_proj = np.asarray(inputs["W_proj"], dtype=np.float32)
    nc = _get_nc()
    in_maps = make_in_maps(x, W_qkv, W_proj)
    res = run_bass_kernel_spmd(nc, in_maps, core_ids=list(range(NCORES)))
    return assemble(res.results)
